# revision 1
# baseline (speedup 1.0000x reference)
"""Trainium2 Bass kernel for nn_MemoryBlock (batched LSTM scan with reset gating).

Problem (hardcoded shapes):
  bs=512, na=64, seq_len=16, nt=32, H=512, N_ATTN=256.
  x = concat(h_self[:,:,256:], h_inter, -1)            -> [512, 64, 512]
  time-major X: [16, 2048, 512]; LSTM cell per step with
  h,c reset-masked by (1-reset) before the cell. Outputs all
  intermediate h,c states, remapped back to [512, 64, 512].

Strategy: data-parallel over the 2048-row step-batch, 256 rows/core on 8
cores; small LSTM weights replicated. All layout transforms (time-major
reorder, feature-major transposes, weight pre-transposition, reset-mask
replication) are done host-side in numpy, so the device kernel is a pure
fused-matmul recurrence:

  per step t: gates.T [2048, 256] = W_comb.T.T @ [x_t; h_{t-1}].T
  accumulated in PSUM over K=1024 (8 chunks of 128: 4 x-chunks then 4
  h-chunks), one PSUM bank per 2 gate-feature chunks (8 banks/step).
  The x-part of step t+1 sits between h-parts of t and t+1 in the PE
  stream, hiding the ACT/DVE cell latency so PE never stalls.

Matmul operands are bf16 (fp32 matmuls are self-loading single-wait
instructions this walrus rejects with Tile's multi-waits, and bf16 enables
fast weight load); PSUM accumulation and all cell math stay fp32
(measured rel err ~2.3e-3 end to end).

Layouts (per core), feature-major "T" = [feature-on-partition, batch]:
  w   [128, 16384] bf16: w[p, (k*16+m)*128+q] = W_comb[128m+q, 128k+p],
                         W_comb = [W_ih | W_hh] (2048 x 1024)
  x   [16, 128, 4, 256] bf16: x[t, p, kc, b] = X[t, row b, 128*kc+p]
  m   [16, 128, 256] bf16: (1-reset) replicated over partitions
  h0,c0 [128, 4, 256] f32: initial states, feature-major
  hys,cys [16, 128, 4, 256] f32 outputs, feature-major (host transposes back)
"""

import sys

import numpy as np

sys.path.insert(0, "/opt/pypackages")
sys.path.insert(0, "/opt/trn_rl_repo")

import concourse.bass as bass  # noqa: E402
import concourse.bacc as bacc  # noqa: E402
import concourse.mybir as mybir  # noqa: E402
import concourse.tile as tile  # noqa: E402

SEQ = 16
NT = 32
NA = 64
H = 512
N_ATTN = 256
BS = NT * SEQ  # 512
BATCH = NT * NA  # 2048
N_CORES = 8
RPC = BATCH // N_CORES  # 256 rows per core
F32 = mybir.dt.float32
BF16 = mybir.dt.bfloat16

_CACHE = {}


def _build_bass():
    """Build the single-core Bass program (same NEFF runs SPMD on 8 cores)."""
    nc = bacc.Bacc(None, target_bir_lowering=False)

    w_d = nc.dram_tensor("w", [128, 8 * 16 * 128], BF16, kind="ExternalInput")
    x_d = nc.dram_tensor("x", [SEQ, 128, 4, 256], BF16, kind="ExternalInput")
    m_d = nc.dram_tensor("m", [SEQ, 128, 256], BF16, kind="ExternalInput")
    h0_d = nc.dram_tensor("h0", [128, 4, 256], F32, kind="ExternalInput")
    c0_d = nc.dram_tensor("c0", [128, 4, 256], F32, kind="ExternalInput")
    hys_d = nc.dram_tensor("hys", [SEQ, 128, 4, 256], F32, kind="ExternalOutput")
    cys_d = nc.dram_tensor("cys", [SEQ, 128, 4, 256], F32, kind="ExternalOutput")

    SIG = mybir.ActivationFunctionType.Sigmoid
    TANH = mybir.ActivationFunctionType.Tanh

    with tile.TileContext(nc) as tc:
        with (
            tc.tile_pool(name="const", bufs=1) as const,
            tc.tile_pool(name="xin", bufs=4) as xin,
            tc.tile_pool(name="min", bufs=4) as min_,
            tc.tile_pool(name="state", bufs=2) as state,
            tc.tile_pool(name="gates", bufs=2) as gpool,
            tc.tile_pool(name="psum", bufs=8, space="PSUM") as psum,
        ):
            # DMA bandwidth is a shared serial resource; emit transfers in
            # consumption order so the first matmuls start after ~1MB, not
            # after the full 4.2MB of weights: x0, W0-3 (x-part of step 0),
            # x1, W4-7 (h-part), then state/masks.
            def load_x(t):
                x4 = xin.tile([128, 4, 256], BF16, tag="x", name=f"x{t}")
                nc.sync.dma_start(x4[:], x_d[t])
                return x4

            w0a = const.tile([128, 4 * 128], BF16, tag="W0a", name="W0a")
            nc.sync.dma_start(w0a[:], w_d[:, 0 : 4 * 128])
            x0p = []
            for q in range(4):
                xp = xin.tile([128, 256], BF16, tag=f"x0p{q}", name=f"x0p{q}",
                              bufs=1)
                nc.sync.dma_start(xp[:], x_d[0, :, q])
                x0p.append(xp)
            x_tiles = {}
            w0b = const.tile([128, 12 * 128], BF16, tag="W0b", name="W0b")
            nc.sync.dma_start(w0b[:], w_d[:, 4 * 128 : 2048])
            Wk = [None] * 8
            for k in range(1, 4):
                wt = const.tile([128, 16 * 128], BF16, tag=f"W{k}", name=f"W{k}")
                nc.sync.dma_start(wt[:], w_d[:, k * 2048 : (k + 1) * 2048])
                Wk[k] = wt
            def load_m(t):
                m = min_.tile([128, 256], BF16, tag="m", name=f"m{t}")
                nc.gpsimd.dma_start(m[:], m_d[t])
                return m

            def load_w(k):
                wt = const.tile([128, 16 * 128], BF16, tag=f"W{k}", name=f"W{k}")
                nc.sync.dma_start(wt[:], w_d[:, k * 2048 : (k + 1) * 2048])
                Wk[k] = wt

            # Initial state, one tile per feature-pair half so every
            # downstream dependency is at half granularity. Interleaved with
            # W4-7 so the h-part weights don't queue behind all of the
            # state/mask bytes in the DMA pipe.
            # SWDGE lanes: any DVE op depending on several of these waits
            # on few sems, staying under walrus's one-sync-wait-per-
            # instruction limit (a DVE "touch" below funnels the mask sem).
            h_prev, c_prev = [], []
            load_w(4)
            for v in range(2):
                hp = state.tile([128, 2, 256], F32, tag=f"h{v}", name=f"h_init{v}")
                cp = state.tile([128, 2, 256], F32, tag=f"c{v}", name=f"c_init{v}")
                nc.gpsimd.dma_start(hp[:], h0_d[:, 2 * v : 2 * v + 2])
                nc.gpsimd.dma_start(cp[:], c0_d[:, 2 * v : 2 * v + 2])
                h_prev.append(hp)
                c_prev.append(cp)
                load_w(5 + v)
            m_tiles = {0: load_m(0)}
            load_w(7)
            m_tiles[1] = load_m(1)
            x_tiles[1] = load_x(1)
            x_tiles[2] = load_x(2)

            def lhsT(k, mi):
                if k == 0:
                    if mi < 4:
                        return w0a[:, mi * 128 : (mi + 1) * 128]
                    return w0b[:, (mi - 4) * 128 : (mi - 3) * 128]
                return Wk[k][:, mi * 128 : (mi + 1) * 128]

            for t in range(SEQ):
                if 3 <= t + 3 < SEQ:
                    x_tiles[t + 3] = load_x(t + 3)
                if 2 <= t + 2 < SEQ:
                    m_tiles[t + 2] = load_m(t + 2)
                if t == 0:
                    xt = [x0p[kc][:] for kc in range(4)]
                else:
                    xt4 = x_tiles.pop(t)
                    xt = [xt4[:, kc, :] for kc in range(4)]
                mt = m_tiles.pop(t)
                m_b = mt[:].unsqueeze(1).broadcast_to([128, 2, 256])

                # Touch mt with a 1-element DVE copy so the mask-muls below
                # never carry two DMA sem waits (walrus allows one sync wait
                # per compute instruction).
                sc = state.tile([128, 1], F32, tag="sc")
                nc.vector.tensor_copy(sc[:], mt[:, :1])

                # Reset-mask previous state (DVE). hm feeds the matmul rhs.
                hm, cm = [], []
                for v in range(2):
                    hmv = state.tile([128, 2, 256], BF16, tag=f"hm{v}",
                                     name=f"hm{t}_{v}")
                    cmv = state.tile([128, 2, 256], F32, tag=f"cm{v}",
                                     name=f"cm{t}_{v}")
                    nc.vector.tensor_mul(hmv[:], h_prev[v][:], m_b)
                    nc.vector.tensor_mul(cmv[:], c_prev[v][:], m_b)
                    hm.append(hmv)
                    cm.append(cmv)

                # 8 PSUM banks: bank j holds gate-feature chunks (2j, 2j+1)
                # for the full 256-row batch -> [128, 2, 256].
                banks = [
                    psum.tile([128, 2, 256], F32, tag="bank", name=f"bank{t}_{j}")
                    for j in range(8)
                ]

                # x-part: K-chunks 0..3 (only needs xt) - PE does this while
                # the previous step's cell math is still in flight.
                for k in range(4):
                    rhs = xt[k][:]
                    for j in range(8):
                        for u in range(2):
                            mi = 2 * j + u
                            # One accumulation group per bank (zero region =
                            # full bank): start only on the bank's first MM.
                            nc.tensor.matmul(
                                banks[j][:, u, :],
                                lhsT(k, mi),
                                rhs,
                                start=(k == 0 and u == 0),
                                stop=False,
                            )

                # h-part: K-chunks 4..7, bank-major (g first, then i, f, o so
                # the cell's critical operands are ready earliest). ACT
                # evacuates each bank into its own per-(gate, half) tile.
                gsb = {}
                for j in (4, 5, 0, 1, 2, 3, 6, 7):
                    for u in range(2):
                        mi = 2 * j + u
                        for k in range(4, 8):
                            kc = k - 4
                            nc.tensor.matmul(
                                banks[j][:, u, :],
                                lhsT(k, mi),
                                hm[kc // 2][:, kc % 2, :],
                                start=False,
                                stop=(k == 7 and u == 1),
                            )
                    # banks 0,1 -> i (sigmoid); 2,3 -> f; 4,5 -> g (tanh);
                    # 6,7 -> o.
                    g_, half = j // 2, j % 2
                    func = TANH if g_ == 2 else SIG
                    gt = gpool.tile([128, 2, 256], F32, tag=f"g{g_}_{half}",
                                    name=f"g{t}_{g_}_{half}")
                    nc.scalar.activation(gt[:], banks[j][:], func)
                    gsb[(g_, half)] = gt

                # Cell math (DVE) + tanh (ACT), independent per half.
                h_new, c_new = [], []
                for v in range(2):
                    ig = state.tile([128, 2, 256], F32, tag=f"ig{v}",
                                    name=f"ig{t}_{v}")
                    nc.vector.tensor_mul(ig[:], gsb[(0, v)][:], gsb[(2, v)][:])
                    cn = state.tile([128, 2, 256], F32, tag=f"c{v}",
                                    name=f"c{t}_{v}")
                    nc.vector.tensor_mul(cn[:], gsb[(1, v)][:], cm[v][:])
                    nc.vector.tensor_add(cn[:], cn[:], ig[:])
                    th = state.tile([128, 2, 256], F32, tag=f"th{v}",
                                    name=f"th{t}_{v}")
                    nc.scalar.activation(th[:], cn[:], TANH)
                    hn = state.tile([128, 2, 256], F32, tag=f"h{v}",
                                    name=f"h{t}_{v}")
                    nc.vector.tensor_mul(hn[:], gsb[(3, v)][:], th[:])
                    nc.sync.dma_start(cys_d[t, :, 2 * v : 2 * v + 2], cn[:])
                    nc.sync.dma_start(hys_d[t, :, 2 * v : 2 * v + 2], hn[:])
                    h_new.append(hn)
                    c_new.append(cn)
                h_prev, c_prev = h_new, c_new

    nc.compile()
    return nc


def _get_nc():
    if "nc" not in _CACHE:
        _CACHE["nc"] = _build_bass()
    return _CACHE["nc"]


def _prep_inputs(h_self, h_inter, hxs, cxs, reset, W_ih, W_hh, b_ih, b_hh):
    """Host-side layout transforms -> list of per-core input dicts."""
    f = np.float32
    x = np.concatenate([h_self[:, :, N_ATTN:], h_inter], axis=-1).astype(f)  # [512,64,512]
    # time-major [16, 2048, 512]
    x_tm = np.ascontiguousarray(
        x.reshape(NT, SEQ, NA, H).transpose(1, 0, 2, 3).reshape(SEQ, BATCH, H)
    )
    resets = np.broadcast_to(reset.astype(f), (BS, NA))
    resets_tm = resets.reshape(NT, SEQ, NA).transpose(1, 0, 2).reshape(SEQ, BATCH)
    mask_tm = (1.0 - resets_tm).astype(f)
    h0 = hxs[::SEQ].reshape(BATCH, H).astype(f)
    c0 = cxs[::SEQ].reshape(BATCH, H).astype(f)

    assert not np.any(b_ih) and not np.any(b_hh), "nonzero LSTM bias unsupported"

    # Weights: W_comb = [W_ih | W_hh] [2048, 1024]; A = W_comb.T [1024, 2048]
    # w[p, (k*16+m)*128+q] = A[128k+p, 128m+q]
    import ml_dtypes
    bf16 = ml_dtypes.bfloat16
    A = np.concatenate([W_ih, W_hh], axis=1).T.astype(f)  # [1024, 2048]
    w = np.ascontiguousarray(
        A.reshape(8, 128, 16, 128).transpose(1, 0, 2, 3).reshape(128, 8 * 16 * 128)
    ).astype(bf16)

    in_maps = []
    for c in range(N_CORES):
        rows = slice(c * RPC, (c + 1) * RPC)
        xc = x_tm[:, rows, :]  # [16, 256, 512]
        # x[t, p, kc, b] = xc[t, b, 128*kc+p]
        xd = np.ascontiguousarray(
            xc.reshape(SEQ, RPC, 4, 128).transpose(0, 3, 2, 1)
        ).astype(bf16)  # [16, 128, 4, 256]
        md = np.ascontiguousarray(
            np.broadcast_to(mask_tm[:, rows][:, None, :], (SEQ, 128, RPC))
        ).astype(bf16)
        h0d = np.ascontiguousarray(h0[rows].reshape(RPC, 4, 128).transpose(2, 1, 0))
        c0d = np.ascontiguousarray(c0[rows].reshape(RPC, 4, 128).transpose(2, 1, 0))
        in_maps.append({"w": w, "x": xd, "m": md, "h0": h0d, "c0": c0d})
    return in_maps


def _postprocess(results):
    """Per-core feature-major outputs -> full [512, 64, 512] hys, cys."""
    outs = []
    for key in ("hys", "cys"):
        tm = np.empty((SEQ, BATCH, H), dtype=np.float32)
        for c, res in enumerate(results):
            rows = slice(c * RPC, (c + 1) * RPC)
            # res[key][t, p, kc, b] -> tm[t, row b, 128*kc+p]
            tm[:, rows, :] = (
                res[key].transpose(0, 3, 2, 1).reshape(SEQ, RPC, H)
            )
        out = tm.reshape(SEQ, NT, NA, H).transpose(1, 0, 2, 3).reshape(BS, NA, H)
        outs.append(np.ascontiguousarray(out))
    return outs[0], outs[1]


def kernel(h_self, h_inter, hxs, cxs, reset, W_ih, W_hh, b_ih, b_hh, seq_len,
           trace=False, tmpdir=None):
    assert int(seq_len) == SEQ
    from concourse.bass_utils import run_bass_kernel_spmd

    nc = _get_nc()
    in_maps = _prep_inputs(
        np.asarray(h_self), np.asarray(h_inter), np.asarray(hxs), np.asarray(cxs),
        np.asarray(reset), np.asarray(W_ih), np.asarray(W_hh),
        np.asarray(b_ih), np.asarray(b_hh),
    )
    res = run_bass_kernel_spmd(
        nc, in_maps, core_ids=list(range(N_CORES)), trace=trace, tmpdir=tmpdir
    )
    _CACHE["last_results"] = res
    return _postprocess(res.results)

